# revision 1
# baseline (speedup 1.0000x reference)
"""DHASPI level-loss kernel for 8 Trainium2 NeuronCores.

Data-parallel over the fused B*C row axis: each core gets 64 rows of x_env
(SBUF partitions 0-63) and 64 rows of y_env (partitions 64-127). The device
computes per-row energies of the 200 non-overlapping 960-sample blocks
(gcd(9600, 2880) — every overlapping loudness frame is a sum of 10 of
them); everything downstream of the block sums — frame energies, the
absolute/relative gating, lufs, the relu-diff loss — is a tiny [1024, 200]
float64 numpy epilogue on the host.

The 200 blocks per row are processed as variable-size work units (1-5
blocks) spread across all four instruction-issuing engines so DMA and
compute overlap fully (raw Bass, manual semaphores):

  route   DMA                    square                block reduction
  C (73b) SP HWDGE f32           ACT Square+accum_out (fused, per block)
  B (18b) Pool SWDGE cast->fp8   ACT Square+accum_out (fused, per block)
  E (89b) Pool SWDGE cast->bf16  DVE mult (bf16, 2x)   Pool accum-DMA fold
                                                       960->480, then one
                                                       DVE reduce_sum
  N (13b) Pool SWDGE cast->bf16  DVE mult (bf16, 2x)   one DVE reduce_sum

Design notes:
- The split sizes the four engine loads to come out nearly equal (~108 us
  each): SP moves f32, Pool halves/quarters its transfer cost by casting to
  bf16/fp8 during the DMA, ACT squares with the fused per-block accumulator,
  and DVE squares at the 2x bf16 rate then reduces with one reduce_sum,
  folded 960->480 beforehand on Pool via an accumulate-DMA for E units.
- fp8(e4m3) input is only used on ACT-fed B units (ACT squares in f32
  regardless of input dtype); measured end-to-end loss error ~1e-3 vs the
  2e-2 tolerance.
- Chained in-place DVE add-tree levels were measured to return stale data
  on the device stochastically (same-engine RAW through the DVE write
  pipe; CoreSim does not model it), so all block reductions are single
  reduce_sum instructions — a sequential reduce never outruns its
  producer — with Pool's accumulate-DMA fold halving the reduce width for
  most units.
- Engine instruction orders are static: the DVE stream weaves N units with
  the E square/finish pipeline, and the Pool stream is deadline-sorted
  against a no-stall relaxation so casts and folds arrive just ahead of
  their consumers.
"""

import numpy as np

import concourse.bass as bass
from concourse import mybir
from concourse.bass_utils import run_bass_kernel_spmd

# Problem constants (hardcoded; kernel.py must be self-contained)
B, C, T = 16, 32, 192000
N_CORES = 8
ROWS = B * C  # 512
RPC = ROWS // N_CORES  # 64 rows per core per tensor

FRAME = 9600
SHIFT = 2880
BLK = 960  # gcd(FRAME, SHIFT)
NBLK = T // BLK  # 200 block sums per row
NFRM = (T - FRAME) // SHIFT + 1  # 64 frames per row

UB = 5  # blocks per work unit
USAMP = UB * BLK  # 4800 samples
NU = NBLK // UB  # 40 units

EPS = 1e-8
ALPHA = 1e-4
GAMMA_A = -70.0

F32 = mybir.dt.float32
BF16 = mybir.dt.bfloat16
FP8 = mybir.dt.float8e4

# Per-route unit sizes in 960-blocks. Small first units cut pipeline fill;
# small last units shorten each engine's tail behind the final transfers.
_FMARGIN = 2500.0  # fold deadline margin ns
_FINGAP = 5300.0  # min ns between sqE_k end and finE_k
C_SIZES = [1, 2, 3] + [4] * 17 + [2, 2]  # 78 blocks, 22 units (SP f32 -> ACT fused)
B_SIZES = [2, 3, 5, 5, 5]  # 20 blocks, 5 units (Pool fp8 -> ACT fused)
# Block reduction is reduce_sum only (see docstring): E units fold
# 960->480 on Pool first, N units reduce the full 960 directly.
E_SIZES = [2, 3] + [5] * 16 + [2, 2]  # 89 blocks (Pool -> DVE sq -> fold -> red480)
N_SIZES = [5, 5, 3]  # 13 blocks (Pool -> DVE sq -> red960)

# Contiguous global block ranges per route: C | B | E | N covers 0..199.
def _spans(sizes, start):
    out = []
    for s in sizes:
        out.append((start, s))
        start += s
    return out, start

C_SPANS, _o = _spans(C_SIZES, 0)
B_SPANS, _o = _spans(B_SIZES, _o)
E_SPANS, _o = _spans(E_SIZES, _o)
N_SPANS, _o = _spans(N_SIZES, _o)
assert _o == NBLK

# SP stream: all C units in order; ACT consumes them in the same order.
SP_STREAM = [("C", i) for i in range(len(C_SIZES))]  # 20 units
# ACT stream: C units as they land, B units filling ACT's spare rate.
ACT_STREAM = (
    [("SP", 0), ("SP", 1), ("B", 0), ("SP", 2), ("SP", 3), ("SP", 4)]
    + [("B", 1)]
    + [("SP", 5), ("SP", 6), ("SP", 7)]
    + [("B", 2)]
    + [("SP", 8), ("SP", 9), ("SP", 10)]
    + [("B", 3)]
    + [("SP", 11), ("SP", 12), ("SP", 13), ("SP", 14)]
    + [("B", 4)]
    + [("SP", 15), ("SP", 16), ("SP", 17), ("SP", 18), ("SP", 19), ("SP", 20)]
    + [("SP", 21)]
)
# DVE stream: N units and the E pipeline woven together from the start —
# sqE early so Pool's folds can fire early; finE only once the fold
# round-trip (fold DMA + completion latency) has plausibly finished.
def _cost_dve(kind, k):
    if kind == "N":
        return 1540.0 * N_SIZES[k] + 200.0
    if kind == "sqE":
        return 524.0 * E_SIZES[k] + 100.0
    return 500.0 * E_SIZES[k] + 100.0


def _build_dve_stream():
    out = []
    ne, nn = len(E_SIZES), len(N_SIZES)
    t = 0.0
    sq_end = {}

    def emit(kind, k):
        nonlocal t
        out.append((kind, k))
        t += _cost_dve(kind, k)
        if kind == "sqE":
            sq_end[k] = t

    emit("sqE", 0)
    emit("sqE", 1)
    ei_sq, ei_fin, ni = 2, 0, 0
    while ei_sq < ne or ei_fin < ne or ni < nn:
        if ni < nn:
            emit("N", ni); ni += 1
        if ei_sq < ne:
            # esq ring: finE_k must precede sqE_{k+NESQ}
            if ei_sq - ei_fin >= 4:
                emit("finE", ei_fin); ei_fin += 1
            emit("sqE", ei_sq); ei_sq += 1
        # fold_k issues after sqE_k lands; needs ~1850 + 2800 to come back
        if ei_fin < ne and (
            (ei_sq >= ne and ni >= nn)
            or (
                ei_sq - ei_fin >= 3
                and ei_fin < ei_sq
                and t >= sq_end[ei_fin] + _FINGAP
            )
        ):
            emit("finE", ei_fin); ei_fin += 1
    return out

DVE_STREAM = _build_dve_stream()

# Pool stream: generated by deadline-sorting casts and folds against a
# no-stall relaxation of the fixed DVE/ACT streams.
def _build_pool_stream():
    LAT = 2800.0  # DMA completion latency (init + sem prop)
    # DVE op begin times under no stalls
    t = 3500.0
    dve_begin = {}
    for kind, k in DVE_STREAM:
        dve_begin[(kind, k)] = t
        t += _cost_dve(kind, k)
    # ACT unit begin times under no stalls
    t = 4100.0
    act_begin = {}
    for kind, k in ACT_STREAM:
        act_begin[(kind, k)] = t
        nb = C_SIZES[k] if kind == "SP" else B_SIZES[k]
        t += 1182.0 * nb + 30.0
    ops = []
    for k in range(len(N_SIZES)):
        ops.append((dve_begin[("N", k)] - LAT, ("cN", k)))
    for k in range(len(E_SIZES)):
        ops.append((dve_begin[("sqE", k)] - LAT, ("cE", k)))
        # fold released only after sqE_k; deadline is finE_k
        rel = dve_begin[("sqE", k)] + 524.0 * E_SIZES[k] + 400.0
        ops.append((max(dve_begin[("finE", k)] - LAT - _FMARGIN, rel), ("f", k)))
    for k in range(len(B_SIZES)):
        ops.append((act_begin[("B", k)] - LAT, ("cB", k)))
    ops.sort(key=lambda x: x[0])
    return [op for _, op in ops]

POOL_STREAM = _build_pool_stream()

NCBUF = 4  # SP f32 ring (4-block tiles)
NBBUF = 3  # Pool->ACT fp8 ring
NEBUF = 4  # Pool->DVE bf16 ring (E)
NESQ = 4  # E squared-tile ring
NNBUF = 3  # Pool->DVE bf16 ring (N)
NNSQ = 2  # N squared-tile ring

CB_BLKS = 4  # C tiles are at most 4 blocks


TREE_LEVELS_FULL = [480, 240, 120, 60, 30, 15]
TREE_LEVELS_HALF = [240, 120, 60, 30, 15]


def _build_program() -> bass.Bass:
    nc = bass.Bass("TRN2", target_bir_lowering=False, debug=False)
    AF = mybir.ActivationFunctionType
    ALU = mybir.AluOpType
    AX = mybir.AxisListType

    xy = nc.dram_tensor("xy", [128, T], F32, kind="ExternalInput").ap()
    out = nc.dram_tensor("bs", [128, NBLK], F32, kind="ExternalOutput").ap()

    cbuf = [
        nc.alloc_sbuf_tensor(f"cb{i}", [128, CB_BLKS * BLK], F32).ap()
        for i in range(NCBUF)
    ]
    bbuf = [nc.alloc_sbuf_tensor(f"bb{i}", [128, USAMP], FP8).ap() for i in range(NBBUF)]
    ebuf = [nc.alloc_sbuf_tensor(f"eb{i}", [128, USAMP], BF16).ap() for i in range(NEBUF)]
    esq = [nc.alloc_sbuf_tensor(f"es{i}", [128, USAMP], BF16).ap() for i in range(NESQ)]
    nbuf = [nc.alloc_sbuf_tensor(f"nb{i}", [128, USAMP], BF16).ap() for i in range(NNBUF)]
    nsq = [nc.alloc_sbuf_tensor(f"ns{i}", [128, USAMP], BF16).ap() for i in range(NNSQ)]
    bs = nc.alloc_sbuf_tensor("bst", [128, NBLK], F32).ap()
    junk = nc.alloc_sbuf_tensor("junk", [128, BLK], BF16).ap()

    def blkview(ap, nb):
        return ap[:, 0 : nb * BLK].rearrange("p (n b) -> p n b", b=BLK)

    sp_spans = list(C_SPANS)

    with (
        nc.Block() as block,
        nc.semaphore("dmaC") as dmaC,
        nc.semaphore("dmaB") as dmaB,
        nc.semaphore("dmaE") as dmaE,
        nc.semaphore("dmaN") as dmaN,
        nc.semaphore("foldE") as foldE,
        nc.semaphore("spfree") as spfree,
        nc.semaphore("bfree") as bfree,
        nc.semaphore("esqs") as esqs,
        nc.semaphore("nsqs") as nsqs,
        nc.semaphore("actfin") as actfin,
        nc.semaphore("dvefin") as dvefin,
        nc.semaphore("outs") as outs,
    ):
        @block.sync
        def _(sync):
            for s, (blk0, nb) in enumerate(sp_spans):
                if s >= NCBUF:
                    sync.wait_ge(spfree, s - NCBUF + 1)
                off = blk0 * BLK
                sync.dma_start(
                    out=cbuf[s % NCBUF][:, 0 : nb * BLK], in_=xy[:, off : off + nb * BLK]
                ).then_inc(dmaC, 16)
            sync.wait_ge(actfin, 1)
            sync.wait_ge(dvefin, 1)
            sync.dma_start(out=out, in_=bs).then_inc(outs, 16)
            sync.wait_ge(outs, 16)

        @block.gpsimd
        def _(g):
            for kind, k in POOL_STREAM:
                if kind == "cB":
                    if k >= NBBUF:
                        g.wait_ge(bfree, k - NBBUF + 1)
                    blk0, nb = B_SPANS[k]
                    off = blk0 * BLK
                    g.dma_start(
                        out=bbuf[k % NBBUF][:, 0 : nb * BLK],
                        in_=xy[:, off : off + nb * BLK],
                    ).then_inc(dmaB, 16)
                elif kind == "cE":
                    if k >= NEBUF:
                        g.wait_ge(esqs, k - NEBUF + 1)
                    blk0, nb = E_SPANS[k]
                    off = blk0 * BLK
                    g.dma_start(
                        out=ebuf[k % NEBUF][:, 0 : nb * BLK],
                        in_=xy[:, off : off + nb * BLK],
                    ).then_inc(dmaE, 16)
                elif kind == "cN":
                    if k >= NNBUF:
                        g.wait_ge(nsqs, k - NNBUF + 1)
                    blk0, nb = N_SPANS[k]
                    off = blk0 * BLK
                    g.dma_start(
                        out=nbuf[k % NNBUF][:, 0 : nb * BLK],
                        in_=xy[:, off : off + nb * BLK],
                    ).then_inc(dmaN, 16)
                else:  # fold: sq tile halves 960 -> 480, in place, accum add
                    g.wait_ge(esqs, k + 1)
                    v = blkview(esq[k % NESQ], E_SPANS[k][1])
                    g.dma_start(
                        out=v[:, :, 0:480], in_=v[:, :, 480:960], accum_op=ALU.add
                    ).then_inc(foldE, 16)

        @block.scalar
        def _(scalar):
            last = len(ACT_STREAM) - 1
            for pos, (kind, k) in enumerate(ACT_STREAM):
                if kind == "SP":
                    scalar.wait_ge(dmaC, 16 * (k + 1))
                    blk0, nb = sp_spans[k]
                    tile = cbuf[k % NCBUF]
                    for b in range(nb):
                        inst = scalar.activation(
                            junk,
                            tile[:, b * BLK : (b + 1) * BLK],
                            AF.Square,
                            accum_out=bs[:, blk0 + b : blk0 + b + 1],
                        )
                    inst.then_inc(spfree, 1)
                else:  # B unit
                    scalar.wait_ge(dmaB, 16 * (k + 1))
                    blk0, nb = B_SPANS[k]
                    tile = bbuf[k % NBBUF]
                    for b in range(nb):
                        inst = scalar.activation(
                            junk,
                            tile[:, b * BLK : (b + 1) * BLK],
                            AF.Square,
                            accum_out=bs[:, blk0 + b : blk0 + b + 1],
                        )
                    inst.then_inc(bfree, 1)
                if pos == last:
                    scalar.drain().then_inc(actfin, 1)

        @block.vector
        def _(vector):
            lp = nc.allow_low_precision

            def tree(v, levels, cols, nb):
                # Chained in-place tree levels are unsafe on the device
                # (same-engine RAW through the DVE write pipe); a single
                # sequential reduce never outruns the producer, so reduce.
                return vector.reduce_sum(cols, v[:, :, 0 : 2 * levels[0]], axis=AX.X)

            last = len(DVE_STREAM) - 1
            for pos, (kind, k) in enumerate(DVE_STREAM):
                if kind == "sqE":
                    vector.wait_ge(dmaE, 16 * (k + 1))
                    nb = E_SPANS[k][1]
                    with lp("bf16 squares"):
                        inst = vector.tensor_tensor(
                            esq[k % NESQ][:, 0 : nb * BLK],
                            ebuf[k % NEBUF][:, 0 : nb * BLK],
                            ebuf[k % NEBUF][:, 0 : nb * BLK],
                            op=ALU.mult,
                        )
                    inst.then_inc(esqs, 1)
                elif kind == "finE":
                    vector.wait_ge(foldE, 16 * (k + 1))
                    blk0, nb = E_SPANS[k]
                    inst = tree(
                        blkview(esq[k % NESQ], nb),
                        TREE_LEVELS_HALF,
                        bs[:, blk0 : blk0 + nb],
                        nb,
                    )
                elif kind == "N":
                    vector.wait_ge(dmaN, 16 * (k + 1))
                    blk0, nb = N_SPANS[k]
                    with lp("bf16 squares"):
                        vector.tensor_tensor(
                            nsq[k % NNSQ][:, 0 : nb * BLK],
                            nbuf[k % NNBUF][:, 0 : nb * BLK],
                            nbuf[k % NNBUF][:, 0 : nb * BLK],
                            op=ALU.mult,
                        ).then_inc(nsqs, 1)
                    inst = tree(
                        blkview(nsq[k % NNSQ], nb),
                        TREE_LEVELS_FULL,
                        bs[:, blk0 : blk0 + nb],
                        nb,
                    )
                if pos == last:
                    inst.then_inc(dvefin, 1)

    return nc


def make_in_maps(x_env: np.ndarray, y_env: np.ndarray) -> list[dict[str, np.ndarray]]:
    x = np.asarray(x_env, dtype=np.float32).reshape(ROWS, T)
    y = np.asarray(y_env, dtype=np.float32).reshape(ROWS, T)
    in_maps = []
    for i in range(N_CORES):
        shard = np.concatenate(
            [x[i * RPC : (i + 1) * RPC], y[i * RPC : (i + 1) * RPC]], axis=0
        )
        in_maps.append({"xy": np.ascontiguousarray(shard)})
    return in_maps


def lufs_from_bs(bs: np.ndarray) -> np.ndarray:
    """Per-row gated lufs from [N, 200] block energy sums (float64 host math)."""
    bs = np.asarray(bs, dtype=np.float64)
    n = bs.shape[0]
    # frame f = blocks 3f..3f+9; cumulative sum gives all frame windows
    cs = np.concatenate([np.zeros((n, 1)), np.cumsum(bs, axis=1)], axis=1)
    starts = 3 * np.arange(NFRM)
    z = (cs[:, starts + 10] - cs[:, starts]) / FRAME  # [N, 64]
    el = -0.691 + 10.0 * np.log10(z + EPS)
    idx_a = (el > GAMMA_A).astype(np.float64)
    z_ave_a = (z * idx_a).sum(1, keepdims=True) / (idx_a.sum(1, keepdims=True) + EPS)
    gamma_r = -0.691 + 10.0 * np.log10(z_ave_a + EPS) - 10.0
    idx_ar = idx_a * (el > gamma_r)
    z_ave_ar = (z * idx_ar).sum(1, keepdims=True) / (idx_ar.sum(1, keepdims=True) + EPS)
    return (-0.691 + 10.0 * np.log10(z_ave_ar + EPS)).reshape(n)


def finish(per_core_bs: list[np.ndarray]) -> np.ndarray:
    total = 0.0
    for bsc in per_core_bs:
        lufs = lufs_from_bs(np.asarray(bsc).reshape(128, NBLK))
        total += np.maximum(lufs[RPC:] - lufs[:RPC], 0.0).sum()
    return np.array(ALPHA * total, dtype=np.float32)


def kernel(x_env: np.ndarray, y_env: np.ndarray) -> np.ndarray:
    nc = _build_program()
    in_maps = make_in_maps(x_env, y_env)
    res = run_bass_kernel_spmd(nc, in_maps, core_ids=list(range(N_CORES)))
    return finish([res.results[i]["bs"] for i in range(N_CORES)])



# revision 45
# speedup vs baseline: 1.0591x; 1.0591x over previous
"""DHASPI level-loss kernel for 8 Trainium2 NeuronCores.

Data-parallel over the fused B*C row axis: each core gets 64 rows of x_env
(SBUF partitions 0-63) and 64 rows of y_env (partitions 64-127). The device
computes per-row energies of the 200 non-overlapping 960-sample blocks
(gcd(9600, 2880) — every overlapping loudness frame is a sum of 10 of
them); everything downstream of the block sums — frame energies, the
absolute/relative gating, lufs, the relu-diff loss — is a tiny [1024, 200]
float64 numpy epilogue on the host.

The 200 blocks per row are split across four routes chosen by a
linear-program balance of the four instruction queues (SP, Pool, ACT, DVE):

  route  load                 square + block reduction
  A (83) Pool SWDGE cast->fp8 ACT Square with fused accum_out  (1172/blk)
  T8(51) Pool SWDGE cast->fp8 DVE tensor_tensor_reduce         (1060/blk)
  T32(26)SP HWDGE f32         DVE tensor_tensor_reduce         (1060/blk)
  P (40) SP HWDGE f32         Pool tensor_tensor square (bf16 out, 800/blk)
                              + Pool accum-DMA fold 960->480 (370/blk)
                              + DVE pair-add 480->240 + reduce_sum 240
                              (~400/blk on DVE)

Design notes:
- tensor_tensor_reduce fuses square and block-sum in one DVE pass (1.10
  ns/elem) — strictly better than the square+fold+reduce pipeline (same
  DVE cost but zero Pool fold traffic), and its f32 [128,1] accumulator
  sidesteps the low-precision reduce penalty.
- fp8(e4m3) input feeds ACT and DVE TTR: both square in f32 internally, so
  the only error is input quantization (~1.8% per sample, averaging out
  over 9600-sample frames; the squaring bias cancels between x and y).
- Same-engine DVE RAW hazards (stale reads through the DVE write pipe on
  real HW) are avoided: TTR reads only DMA-written tiles; the P-route
  pair-add reads Pool-fold output (cross-engine, semaphore-gated) and the
  following reduce reads the pair-add output no faster than it was
  written (sequential reduce never outruns its producer).
- Engine instruction orders are static, generated by a latest-start-time
  list scheduler over the route dependency graph so producers land just
  ahead of their consumers.
"""

import numpy as np

import concourse.bass as bass
from concourse import mybir
from concourse.bass_utils import run_bass_kernel_spmd

# Problem constants (hardcoded; kernel.py must be self-contained)
B, C, T = 16, 32, 192000
N_CORES = 8
ROWS = B * C  # 512
RPC = ROWS // N_CORES  # 64 rows per core per tensor

FRAME = 9600
SHIFT = 2880
BLK = 960  # gcd(FRAME, SHIFT)
NBLK = T // BLK  # 200 block sums per row
NFRM = (T - FRAME) // SHIFT + 1  # 64 frames per row

EPS = 1e-8
ALPHA = 1e-4
GAMMA_A = -70.0

F32 = mybir.dt.float32
BF16 = mybir.dt.bfloat16
FP8 = mybir.dt.float8e4

# Route unit sizes in 960-blocks (LP-balanced: A=83, T8=51, T32=26, P=40).
# Small first units cut pipeline fill.
A_SIZES = [1, 3] + [5] * 16  # 84 blocks -> ACT square+accum
T8_SIZES = [1, 3] + [5] * 9  # 49 blocks -> DVE bn_stats (fp8)
T32_SIZES = [2, 3, 4, 4, 4, 3, 3, 1]  # 24 blocks -> DVE bn_stats (f32)
P_SIZES = [1, 2] + [5] * 7 + [4, 1]  # 43 blocks -> Pool sq + fold + DVE add/red


def _spans(sizes, start):
    out = []
    for s in sizes:
        out.append((start, s))
        start += s
    return out, start


A_SPANS, _o = _spans(A_SIZES, 0)
T8_SPANS, _o = _spans(T8_SIZES, _o)
T32_SPANS, _o = _spans(T32_SIZES, _o)
P_SPANS, _o = _spans(P_SIZES, _o)
assert _o == NBLK

NABUF = 5  # fp8 ring for A route
NT8BUF = 5  # fp8 ring for T8 route
NT32BUF = 3  # f32 ring for T32 route
NPBUF = 4  # f32 ring for P route
NSQBUF = 3  # bf16 squared ring for P route

MAXU_A = max(A_SIZES) * BLK
MAXU_T8 = max(T8_SIZES) * BLK
MAXU_T32 = max(T32_SIZES) * BLK
MAXU_P = max(P_SIZES) * BLK

# ---------------------------------------------------------------------------
# Static scheduling: build one op list with dependencies and durations,
# list-schedule by latest-start-time, and use the per-engine order for
# emission.  Durations are V1-cost-model estimates in ns.

_DMA_LAT = 5500.0  # DMA init + completion-sem propagation + margin


def _dur(kind, nb):
    return {
        "cA": 370.6 * nb,
        "cT8": 370.6 * nb,
        "cT32": 1480.5 * nb,
        "cP": 1480.5 * nb,
        "sqP": 800.0 * nb,
        "f1P": max(370.6 * nb, 500.0),
        "actA": 1172.0 * nb,
        "ttr8": 1120.0 * nb,
        "ttr32": 1120.0 * nb,
        "redP": 400.0 * nb,
    }[kind]


def _schedule():
    """Three-pass schedule.

    1. Draft consumer streams (ACT sequential; DVE by target fractions),
       deadline-sort producers, and serially estimate SP/Pool timelines
       with release clamping.
    2. Rebuild the DVE stream by list-scheduling against estimated tile
       ready-times, so no DVE op sits in the stream before its data can
       exist.
    3. Re-sort producers against the final consumer need-times.
    """
    n8, n32, nP = len(T8_SPANS), len(T32_SPANS), len(P_SPANS)
    kind_spans = {"ttr8": T8_SPANS, "ttr32": T32_SPANS, "redP": P_SPANS}

    act_order = [("actA", k) for k in range(len(A_SPANS))]

    def act_need_of():
        need, t = {}, 0.0
        for kind, k in act_order:
            need[(kind, k)] = t
            t += _dur("actA", A_SPANS[k][1])
        return need

    act_need = act_need_of()

    def dve_need_of(order):
        need, t = {}, 0.0
        for kind, k in order:
            need[(kind, k)] = t
            t += _dur(kind, kind_spans[kind][k][1])
        return need, t

    # Pass 1 draft DVE order: fraction merge.
    units = []
    for k in range(n8):
        units.append(((k + 0.25) / n8, ("ttr8", k)))
    for k in range(n32):
        units.append(((k + 0.75) / n32, ("ttr32", k)))
    for k in range(nP):
        units.append(((k + 1.1) / (nP + 1.45), ("redP", k)))
    units.sort(key=lambda x: x[0])
    dve_order = [u for _, u in units]

    def producer_streams(dve_need, pool_done_prev=None):
        # consumer completion estimates (for ring-free releases)
        dve_done = {
            u: t + _dur(u[0], kind_spans[u[0]][u[1]][1])
            for u, t in dve_need.items()
        }
        act_done = {
            u: t + _dur("actA", A_SPANS[u[1]][1]) for u, t in act_need.items()
        }
        sem = 400.0  # engine-side sem propagation to the producer

        sp_ops = []
        for k in range(n32):
            rel = (
                dve_done[("ttr32", k - NT32BUF)] + sem if k >= NT32BUF else 0.0
            )
            dl = dve_need[("ttr32", k)] - _DMA_LAT
            sp_ops.append((dl, rel, ("cT32", k)))
        for k in range(nP):
            nb = P_SPANS[k][1]
            sq_dl = dve_need[("redP", k)] - _DMA_LAT - _dur("f1P", nb)
            rel = 0.0
            if k >= NPBUF and pool_done_prev is not None:
                rel = pool_done_prev.get(("sqP", k - NPBUF), 2900.0) + sem - 2600.0
            sp_ops.append((sq_dl - _DMA_LAT - _dur("sqP", nb), rel, ("cP", k)))
        sp_ops.sort(key=lambda x: x[0])
        t = 300.0
        sp_done = {}
        for _, rel, op in sp_ops:
            kind, k = op
            t = max(t, rel) + _dur(
                kind, (T32_SPANS if kind == "cT32" else P_SPANS)[k][1]
            )
            sp_done[op] = t + 2600.0
        pool_ops = []
        for k in range(len(A_SPANS)):
            rel = act_done[("actA", k - NABUF)] + sem if k >= NABUF else 0.0
            dl = act_need[("actA", max(k - 1, 0))] - _DMA_LAT
            pool_ops.append((dl, rel, ("cA", k)))
        for k in range(n8):
            rel = dve_done[("ttr8", k - NT8BUF)] + sem if k >= NT8BUF else 0.0
            pool_ops.append(
                (dve_need[("ttr8", k)] - _DMA_LAT - 1.0, rel, ("cT8", k))
            )
        for k in range(nP):
            nb = P_SPANS[k][1]
            f1_dl = dve_need[("redP", k)] - _DMA_LAT
            sq_dl = f1_dl - _dur("f1P", nb)
            rel = sp_done[("cP", k)]
            if k >= NSQBUF:
                rel = max(rel, dve_done[("redP", k - NSQBUF)] + sem)
            pool_ops.append((max(sq_dl, rel), rel, ("sqP", k)))
            pool_ops.append(
                (max(f1_dl, rel + _dur("sqP", nb)), rel, ("f1P", k))
            )
        pool_ops.sort(key=lambda x: x[0])
        t = 300.0
        pool_done = {}
        for _, rel, op in pool_ops:
            kind, k = op
            nb = (
                A_SPANS[k][1]
                if kind == "cA"
                else T8_SPANS[k][1]
                if kind == "cT8"
                else P_SPANS[k][1]
            )
            t = max(t, rel) + _dur(kind, nb)
            pool_done[op] = t + 2600.0
        return (
            [op for _, _, op in sp_ops],
            [op for _, _, op in pool_ops],
            sp_done,
            pool_done,
        )

    dve_need, dve_total = dve_need_of(dve_order)

    # Passes 2..N: alternate (a) producer timelines from current DVE order
    # and (b) DVE re-list-scheduled against tile ready-times, to a fixed
    # point; then a final producer re-sort.
    pool_done = None
    for it in range(4):
        if it < 3:
            sp_order, pool_order, sp_done, pool_done = producer_streams(
                dve_need, pool_done
            )
        # else: final pass — freeze producers, re-derive only the DVE order
        ready = {}
        for k in range(n8):
            ready[("ttr8", k)] = pool_done[("cT8", k)] + 2000.0
        for k in range(n32):
            ready[("ttr32", k)] = sp_done[("cT32", k)] + 2000.0
        for k in range(nP):
            ready[("redP", k)] = (
                pool_done[("f1P", k)] + 2000.0
                if k < nP - 3
                else pool_done[("f1P", min(k + 1, nP - 1))] + 1000.0
            )
        pend = set(ready)
        # within a kind, units must stay in k order: only the lowest
        # unscheduled k of each kind is eligible.
        ptr = {"ttr8": 0, "ttr32": 0, "redP": 0}
        now = 0.0
        new_order = []
        while pend:
            elig = [
                (kind, ptr[kind])
                for kind in ptr
                if (kind, ptr[kind]) in pend
            ]
            ready_now = [u for u in elig if ready[u] <= now]
            pick = (
                min(ready_now, key=lambda u: ready[u])
                if ready_now
                else min(elig, key=lambda u: ready[u])
            )
            now = max(now, ready[pick]) + _dur(
                pick[0], kind_spans[pick[0]][pick[1]][1]
            )
            new_order.append(pick)
            ptr[pick[0]] += 1
            pend.remove(pick)
        dve_order = new_order
        dve_need, dve_total = dve_need_of(dve_order)

    # f1P_k's emission waits on sqP_{min(k+1, nP-1)} (drain fence): enforce
    # that queue order or the Pool queue deadlocks on its own wait.
    reordered = []
    pending = {}  # required sq index -> [fold ops]
    seen_sq = -1
    for op in pool_order:
        kind, k = op
        if kind == "f1P":
            need = min(k + 1, nP - 1)
            if need > seen_sq:
                pending.setdefault(need, []).append(op)
                continue
            reordered.append(op)
        else:
            reordered.append(op)
            if kind == "sqP":
                seen_sq = k
                for j in sorted(list(pending)):
                    if j <= seen_sq:
                        reordered.extend(pending.pop(j))
    for j in sorted(list(pending)):
        reordered.extend(pending.pop(j))
    pool_order = reordered

    order = {
        "SP": sp_order,
        "Pool": pool_order,
        "ACT": act_order,
        "DVE": dve_order,
    }
    for eng, seq in order.items():
        by_kind = {}
        for i, (kind, k) in enumerate(seq):
            by_kind.setdefault(kind, []).append((i, k))
        for kind, pairs in by_kind.items():
            ks = sorted(k for _, k in pairs)
            for (i, _), k in zip(pairs, ks):
                seq[i] = (kind, k)
    return order, dve_total


_ORDER, _EST_FINISH = _schedule()


def _build_program() -> bass.Bass:
    nc = bass.Bass("TRN2", target_bir_lowering=False, debug=False)
    nc._op_labels = {}

    def tag(inst, label):
        nc._op_labels[inst.ins.name] = label
        return inst

    AF = mybir.ActivationFunctionType
    ALU = mybir.AluOpType
    AX = mybir.AxisListType

    xy = nc.dram_tensor("xy", [128, T], F32, kind="ExternalInput").ap()
    out = nc.dram_tensor("bs", [128, NBLK], F32, kind="ExternalOutput").ap()
    out_st = nc.dram_tensor(
        "stats", [128, (sum(T8_SIZES) + sum(T32_SIZES)) * 12], F32,
        kind="ExternalOutput",
    ).ap()

    abuf = [
        nc.alloc_sbuf_tensor(f"ab{i}", [128, MAXU_A], FP8).ap() for i in range(NABUF)
    ]
    t8buf = [
        nc.alloc_sbuf_tensor(f"t8b{i}", [128, MAXU_T8], FP8).ap()
        for i in range(NT8BUF)
    ]
    t32buf = [
        nc.alloc_sbuf_tensor(f"t32b{i}", [128, MAXU_T32], F32).ap()
        for i in range(NT32BUF)
    ]
    pbuf = [
        nc.alloc_sbuf_tensor(f"pb{i}", [128, MAXU_P], F32).ap() for i in range(NPBUF)
    ]
    sqbuf = [
        nc.alloc_sbuf_tensor(f"sq{i}", [128, MAXU_P], BF16).ap()
        for i in range(NSQBUF)
    ]
    bs = nc.alloc_sbuf_tensor("bst", [128, NBLK], F32).ap()
    junkA = nc.alloc_sbuf_tensor("junkA", [128, BLK], BF16).ap()
    nbn = sum(T8_SIZES) + sum(T32_SIZES)  # 73 bn blocks
    stats = nc.alloc_sbuf_tensor("stats_sb", [128, nbn * 12], F32).ap()
    BN_BASE = A_SPANS[-1][0] + A_SPANS[-1][1]  # first bn block index (84)

    with (
        nc.Block() as block,
        nc.semaphore("dmaA") as dmaA,
        nc.semaphore("dmaT8") as dmaT8,
        nc.semaphore("dmaT32") as dmaT32,
        nc.semaphore("dmaP") as dmaP,
        nc.semaphore("foldP") as foldP,
        nc.semaphore("aFree") as aFree,
        nc.semaphore("t8Free") as t8Free,
        nc.semaphore("t32Free") as t32Free,
        nc.semaphore("pf32Free") as pf32Free,
        nc.semaphore("sqFree") as sqFree,
        nc.semaphore("actfin") as actfin,
        nc.semaphore("dvefin") as dvefin,
        nc.semaphore("warm") as warm,
        nc.semaphore("outs") as outs,
    ):
        @block.sync
        def _(sync):
            for key in _ORDER["SP"]:
                kind, k = key
                if kind == "cT32":
                    if k >= NT32BUF:
                        sync.wait_ge(t32Free, k - NT32BUF + 1)
                    b0, nb = T32_SPANS[k]
                    tag(sync.dma_start(
                        out=t32buf[k % NT32BUF][:, 0 : nb * BLK],
                        in_=xy[:, b0 * BLK : (b0 + nb) * BLK],
                    ), ("cT32", k)).then_inc(dmaT32, 16)
                else:  # cP
                    if k >= NPBUF:
                        sync.wait_ge(pf32Free, k - NPBUF + 1)
                    b0, nb = P_SPANS[k]
                    tag(sync.dma_start(
                        out=pbuf[k % NPBUF][:, 0 : nb * BLK],
                        in_=xy[:, b0 * BLK : (b0 + nb) * BLK],
                    ), ("cP", k)).then_inc(dmaP, 16)
            nA = A_SPANS[-1][0] + A_SPANS[-1][1]
            p0 = P_SPANS[0][0]
            sync.wait_ge(actfin, 1)
            sync.dma_start(out=out[:, 0:nA], in_=bs[:, 0:nA]).then_inc(outs, 16)
            sync.wait_ge(dvefin, 1)
            sync.dma_start(
                out=out[:, p0:NBLK], in_=bs[:, p0:NBLK]
            ).then_inc(outs, 16)
            sync.dma_start(out=out_st, in_=stats).then_inc(outs, 16)
            sync.wait_ge(outs, 48)

        @block.gpsimd
        def _(g):
            n_cA = len(A_SPANS) - 1
            n_f1 = len(P_SPANS) - 1
            for key in _ORDER["Pool"]:
                kind, k = key
                if kind == "cA":
                    if k >= NABUF:
                        g.wait_ge(aFree, k - NABUF + 1)
                    b0, nb = A_SPANS[k]
                    tag(g.dma_start(
                        out=abuf[k % NABUF][:, 0 : nb * BLK],
                        in_=xy[:, b0 * BLK : (b0 + nb) * BLK],
                    ), ("cA", k)).then_inc(dmaA, 16)
                    if k == n_cA:
                        # tail dummy: its 16 chunk-completions certify every
                        # DMA engine has drained the last real cA transfer
                        g.dma_start(
                            out=junkA[:, 128:256], in_=xy[:, 0:128]
                        ).then_inc(dmaA, 16)
                elif kind == "cT8":
                    if k >= NT8BUF:
                        g.wait_ge(t8Free, k - NT8BUF + 1)
                    b0, nb = T8_SPANS[k]
                    tag(g.dma_start(
                        out=t8buf[k % NT8BUF][:, 0 : nb * BLK],
                        in_=xy[:, b0 * BLK : (b0 + nb) * BLK],
                    ), ("cT8", k)).then_inc(dmaT8, 16)
                elif kind == "sqP":
                    g.wait_ge(dmaP, 16 * (k + 1))
                    if k >= NSQBUF:
                        g.wait_ge(sqFree, k - NSQBUF + 1)
                    b0, nb = P_SPANS[k]
                    tag(g.tensor_tensor(
                        sqbuf[k % NSQBUF][:, 0 : nb * BLK],
                        pbuf[k % NPBUF][:, 0 : nb * BLK],
                        pbuf[k % NPBUF][:, 0 : nb * BLK],
                        op=ALU.mult,
                    ), ("sqP", k)).then_inc(pf32Free, 1)
                else:  # f1P: fold 960 -> 480, in place, accum add
                    g.wait_ge(pf32Free, min(k + 2, len(P_SPANS)))
                    b0, nb = P_SPANS[k]
                    v = (
                        sqbuf[k % NSQBUF][:, 0 : nb * BLK]
                        .rearrange("p (n b) -> p n b", b=BLK)
                    )
                    tag(g.dma_start(
                        out=v[:, :, 0:480], in_=v[:, :, 480:960], accum_op=ALU.add
                    ), ("f1P", k)).then_inc(foldP, 16)
                    if k == n_f1:
                        g.dma_start(
                            out=junkA[:, 256:320], in_=junkA[:, 320:384],
                            accum_op=ALU.add,
                        ).then_inc(foldP, 16)

        @block.scalar
        def _(scalar):
            # Warm the Square activation table during pipeline fill so the
            # 1383ns table load is off the first real unit's critical path.
            # (Input is a DVE-zeroed scratch; the output is never consumed.)
            scalar.wait_ge(warm, 1)
            scalar.activation(junkA[:, 8:16], junkA[:, 0:8], AF.Square)
            last = len(_ORDER["ACT"]) - 1
            for pos, key in enumerate(_ORDER["ACT"]):
                _, k = key
                scalar.wait_ge(dmaA, 16 * min(k + 2, len(A_SPANS) + 1))
                b0, nb = A_SPANS[k]
                tile = abuf[k % NABUF]
                for b in range(nb):
                    inst = tag(scalar.activation(
                        junkA,
                        tile[:, b * BLK : (b + 1) * BLK],
                        AF.Square,
                        accum_out=bs[:, b0 + b : b0 + b + 1],
                    ), ("actA", k, b))
                inst.then_inc(aFree, 1)
                if pos == last:
                    scalar.drain().then_inc(actfin, 1)

        @block.vector
        def _(vector):
            vector.memset(junkA[:, 0:8], 0.0).then_inc(warm, 1)
            for pos, key in enumerate(_ORDER["DVE"]):
                kind, k = key
                if kind in ("ttr8", "ttr32"):
                    if kind == "ttr8":
                        vector.wait_ge(dmaT8, 16 * (k + 1))
                        b0, nb = T8_SPANS[k]
                        tile = t8buf[k % NT8BUF]
                        free_sem = t8Free
                    else:
                        vector.wait_ge(dmaT32, 16 * (k + 1))
                        b0, nb = T32_SPANS[k]
                        tile = t32buf[k % NT32BUF]
                        free_sem = t32Free
                    for b in range(nb):
                        slot = b0 + b - BN_BASE
                        for c in range(2):
                            inst = tag(vector.bn_stats(
                                stats[:, slot * 12 + c * 6 : slot * 12 + (c + 1) * 6],
                                tile[:, b * BLK + c * 480 : b * BLK + (c + 1) * 480],
                            ), (kind, k, b, c))
                    inst.then_inc(free_sem, 1)
                else:  # redP: pair-add 480->240 then segmented reduce
                    nP_ = len(P_SPANS)
                    w = 16 * (k + 1) if k < nP_ - 3 else 16 * min(k + 2, nP_ + 1)
                    vector.wait_ge(foldP, w)
                    b0, nb = P_SPANS[k]
                    v = (
                        sqbuf[k % NSQBUF][:, 0 : nb * BLK]
                        .rearrange("p (n b) -> p n b", b=BLK)
                    )
                    with nc.allow_low_precision("bf16 pair-add"):
                        tag(vector.tensor_tensor(
                            v[:, :, 480:720],
                            v[:, :, 0:240],
                            v[:, :, 240:480],
                            op=mybir.AluOpType.add,
                        ), ("addP", k))
                    inst = tag(vector.reduce_sum(
                        bs[:, b0 : b0 + nb], v[:, :, 480:720], axis=AX.X
                    ), ("redP", k))
                    inst.then_inc(sqFree, 1)
            vector.memset(junkA[:, 16:24], 0.0).then_inc(dvefin, 1)

    return nc


def make_in_maps(x_env: np.ndarray, y_env: np.ndarray) -> list[dict[str, np.ndarray]]:
    x = np.asarray(x_env, dtype=np.float32).reshape(ROWS, T)
    y = np.asarray(y_env, dtype=np.float32).reshape(ROWS, T)
    in_maps = []
    for i in range(N_CORES):
        shard = np.concatenate(
            [x[i * RPC : (i + 1) * RPC], y[i * RPC : (i + 1) * RPC]], axis=0
        )
        in_maps.append({"xy": np.ascontiguousarray(shard)})
    return in_maps


def assemble_bs(bs: np.ndarray, stats: np.ndarray) -> np.ndarray:
    """Fill the bn-route block sums (columns 84..156) from the device
    bn_stats output: per 480-chunk, sum(x^2) = M2_even + 240*mean_even^2 +
    M2_odd + 240*mean_odd^2; block = chunk0 + chunk1."""
    bs = np.array(bs, dtype=np.float64).reshape(128, NBLK)
    nbn = sum(T8_SIZES) + sum(T32_SIZES)
    st = np.asarray(stats, dtype=np.float64).reshape(128, nbn, 4, 3)
    ss = st[..., 2] + 240.0 * st[..., 1] ** 2  # [128, nbn, 4]
    b0 = A_SPANS[-1][0] + A_SPANS[-1][1]
    bs[:, b0 : b0 + nbn] = ss.sum(axis=2)
    return bs


def lufs_from_bs(bs: np.ndarray) -> np.ndarray:
    """Per-row gated lufs from [N, 200] block energy sums (float64 host math)."""
    bs = np.asarray(bs, dtype=np.float64)
    n = bs.shape[0]
    # frame f = blocks 3f..3f+9; cumulative sum gives all frame windows
    cs = np.concatenate([np.zeros((n, 1)), np.cumsum(bs, axis=1)], axis=1)
    starts = 3 * np.arange(NFRM)
    z = (cs[:, starts + 10] - cs[:, starts]) / FRAME  # [N, 64]
    el = -0.691 + 10.0 * np.log10(z + EPS)
    idx_a = (el > GAMMA_A).astype(np.float64)
    z_ave_a = (z * idx_a).sum(1, keepdims=True) / (idx_a.sum(1, keepdims=True) + EPS)
    gamma_r = -0.691 + 10.0 * np.log10(z_ave_a + EPS) - 10.0
    idx_ar = idx_a * (el > gamma_r)
    z_ave_ar = (z * idx_ar).sum(1, keepdims=True) / (idx_ar.sum(1, keepdims=True) + EPS)
    return (-0.691 + 10.0 * np.log10(z_ave_ar + EPS)).reshape(n)


def finish(per_core: list[tuple[np.ndarray, np.ndarray]]) -> np.ndarray:
    total = 0.0
    for bsc, stc in per_core:
        lufs = lufs_from_bs(assemble_bs(bsc, stc))
        total += np.maximum(lufs[RPC:] - lufs[:RPC], 0.0).sum()
    return np.array(ALPHA * total, dtype=np.float32)


def kernel(x_env: np.ndarray, y_env: np.ndarray) -> np.ndarray:
    nc = _build_program()
    in_maps = make_in_maps(x_env, y_env)
    res = run_bass_kernel_spmd(nc, in_maps, core_ids=list(range(N_CORES)))
    return finish(
        [(res.results[i]["bs"], res.results[i]["stats"]) for i in range(N_CORES)]
    )
